# revision 5
# baseline (speedup 1.0000x reference)
"""DIN attention-unit kernel for Trainium2 (8 NeuronCores, data-parallel over batch).

Math (per batch element b, N=200 items, K=32, HID=36):
    h1[n] = [k_b, q_{b,n}, k_b (x) q_{b,n}]           # [1088]
    act   = sigmoid(h1 @ W1 + b1)                     # [N, 36]
    w     = act @ W2 + b2                             # [N, 1]
    out_b = sum_n q_{b,n} * w[n]                      # [32]

Factorization used on device: since (k (x) q) . W1o = q . (k . W1o_reshaped),
precompute per-b effective weights CD_b = [[C_b],[d_b]] with
    C_b = W1q + k_b . W1o   ([32, 36]),   d_b = k_b . W1k + b1   ([36])
so act[n] = sigmoid([q_n, 1] @ CD_b). Then with M_b = [q|1]^T @ [act|1]
([32, 37] contracted over n), out_b = M_b @ [W2; b2].

Device dataflow per core (128 b's, processed as 64 pairs):
  phase 1:  one matmul  candT_ones^T @ W_pack -> CD for all b  -> DRAM scratch
  per pair: act-mm  (stationary = stacked goodsT+ones [66, nc],
                     rhs = block-diag CD pair [66, 72])  -> act natural [nc, 72]
            sigmoid (ACT engine, full-ish lanes)
            M-mm    (stationary = pair-packed natural goods [nc, 64],
                     rhs = [act|1] pair [nc, 74]) -> PSUM [64, 74] blocks
  epilogue: batched DVE multiply by masked/replicated [W2; b2] + reduce -> out
Goods ships in fp16 twice (natural pair-packed + transposed-with-ones);
all contractions accumulate in fp32 PSUM. End-to-end rel err ~3e-4.
"""

import sys

import numpy as np

if "/opt/trn_rl_repo" not in sys.path:
    sys.path.insert(0, "/opt/trn_rl_repo")

B, N, K, H = 1024, 200, 32, 36
NCORES = 8
BL = B // NCORES          # 128 batch elements per core
PAIRS = BL // 2           # 64
NC2 = N // 2              # 100, n-chunk size
CD_W = 2 * H              # 72  (pair block-diag CD width)
ACT_W = 2 * 37            # 74  (act|ones pair width)
MT = 8                    # m-tiles per core (16 b's each)
F16 = "float16"


def _host_pack(candidate_ad, goods, W1, b1, W2, b2):
    f16 = np.float16
    goods16 = goods.astype(f16)                        # [B, N, K]

    # transposed goods + ones row: gt[b, j, n]; j=32 row is ones
    gt = np.empty((B, K + 1, N), dtype=f16)
    gt[:, :K, :] = goods16.transpose(0, 2, 1)
    gt[:, K, :] = f16(1.0)

    # natural goods, pair-packed on the k axis: gn[p, n, 32c+k] = goods[2p+c, n, k]
    gn = np.empty((B // 2, N, 2 * K), dtype=f16)
    gn[:, :, :K] = goods16[0::2]
    gn[:, :, K:] = goods16[1::2]

    # candT with ones row: [33, B]
    candT1 = np.concatenate(
        [candidate_ad.T, np.ones((1, B), np.float32)], axis=0
    ).astype(np.float32)

    # W_pack [33, 1188]: row i (i<32): cols j*36+h = W1[2K + i*K + j, h]; cols 1152+h = W1[i, h]
    #                    row 32:      cols j*36+h = W1[K + j, h];        cols 1152+h = b1[h]
    wpack = np.empty((K + 1, (K + 1) * H), np.float32)
    W1o = W1[2 * K:].reshape(K, K, H)                  # [i, j, h]
    wpack[:K, : K * H] = W1o.transpose(0, 1, 2).reshape(K, K * H)
    wpack[K, : K * H] = W1[K: 2 * K].reshape(K * H)
    wpack[:K, K * H:] = W1[:K]
    wpack[K, K * H:] = b1
    # CD_cat[b] = candT1[:, b] @ wpack -> reshape [33, 36] = [[C_b (j-major)], [d_b]]

    # masked replicated [W2; b2] for the DVE epilogue: [128, 296]
    w2b2 = np.concatenate([W2[:, 0], b2]).astype(np.float32)   # [37]
    base = np.zeros((64, ACT_W), np.float32)
    for c in range(2):
        base[32 * c: 32 * (c + 1), 37 * c: 37 * (c + 1)] = w2b2[None, :]
    w2m = np.tile(base, (2, 4))                        # [128, 296]
    return gt, gn, candT1, wpack, w2m


def _build_nc():
    import concourse.bacc as bacc
    import concourse.tile as tile
    from concourse import mybir

    f16 = mybir.dt.float16
    f32 = mybir.dt.float32

    nc = bacc.Bacc()
    gt_d = nc.dram_tensor("gt", [BL, K + 1, N], f16, kind="ExternalInput")
    gn_d = nc.dram_tensor("gn", [PAIRS, N, 2 * K], f16, kind="ExternalInput")
    candT_d = nc.dram_tensor("candT", [K + 1, BL], f32, kind="ExternalInput")
    wpack_d = nc.dram_tensor("wpack", [K + 1, (K + 1) * H], f32, kind="ExternalInput")
    w2m_d = nc.dram_tensor("w2m", [128, 4 * ACT_W], f32, kind="ExternalInput")
    out_d = nc.dram_tensor("out", [BL, K], f32, kind="ExternalOutput")

    CDW = (K + 1) * H  # 1188

    with tile.TileContext(nc) as tc:
        with (
            tc.tile_pool(name="const", bufs=1) as constp,
            tc.tile_pool(name="dram", bufs=1, space="DRAM") as dramp,
        ):
            wpack_sb = constp.tile([K + 1, CDW], f32)
            nc.sync.dma_start(wpack_sb[:], wpack_d[:])
            candT_sb = constp.tile([K + 1, BL], f32)
            nc.sync.dma_start(candT_sb[:], candT_d[:])
            w2m_sb = constp.tile([128, 4 * ACT_W], f32)
            nc.sync.dma_start(w2m_sb[:], w2m_d[:])

            # ---- phase 1: CD for all 128 b's, then park in DRAM scratch ----
            c_dram = dramp.tile([BL, CDW], f16)
            with tc.tile_pool(name="ph1psum", bufs=1, space="PSUM") as ph1p:
                c_ps = ph1p.tile([BL, CDW], f32)
                for lo, hi in ((0, 512), (512, 1024), (1024, CDW)):
                    nc.tensor.matmul(
                        c_ps[:, lo:hi], candT_sb[:], wpack_sb[:, lo:hi],
                        start=True, stop=True,
                    )
                c_sb = constp.tile([BL, CDW], f16)
                nc.vector.tensor_copy(c_sb[:], c_ps[:])
            nc.sync.dma_start(c_dram[:], c_sb[:])

            with (
                tc.tile_pool(name="gtp", bufs=3) as gtp,
                tc.tile_pool(name="gnp", bufs=3) as gnp,
                tc.tile_pool(name="cdp", bufs=1) as cdp,
                tc.tile_pool(name="actp", bufs=1) as actp,
                tc.tile_pool(name="prodp", bufs=2) as prodp,
                tc.tile_pool(name="outp", bufs=2) as outp,
                tc.tile_pool(name="actps", bufs=4, space="PSUM") as actps,
                tc.tile_pool(name="mps", bufs=2, space="PSUM") as mps,
            ):
                # persistent zero-background CD tiles (diag blocks re-written per pair)
                cd_tiles = []
                for i in range(4):
                    t = cdp.tile([2 * (K + 1), CD_W], f16, tag=f"cd{i}")
                    nc.vector.memset(t[:], 0.0)
                    cd_tiles.append(t)
                # persistent act tiles with preset ones columns
                act_tiles = []
                for i in range(8):
                    t = actp.tile([NC2, ACT_W], f16, tag=f"act{i}")
                    tv = t.rearrange("p (c e) -> p c e", e=37)
                    nc.vector.memset(tv[:, :, 36:37], 1.0)
                    act_tiles.append(t)

                for mt in range(MT):
                    m_ps = mps.tile([128, 4 * ACT_W], f32)
                    for pq in range(PAIRS // MT):
                        p = (PAIRS // MT) * mt + pq
                        b0 = 2 * p
                        gam = p % 2
                        psi = (p // 2) % 4

                        gt2 = gtp.tile([2 * (K + 1), N], f16)
                        nc.sync.dma_start(
                            gt2[:], gt_d[b0: b0 + 2].rearrange("b j n -> (b j) n")
                        )
                        cd = cd_tiles[p % 4]
                        for c in range(2):
                            nc.sync.dma_start(
                                cd[33 * c: 33 * (c + 1), 36 * c: 36 * (c + 1)],
                                c_dram[b0 + c: b0 + c + 1].rearrange(
                                    "b (j h) -> (b j) h", h=H
                                ),
                            )
                        gn2 = gnp.tile([NC2, 2, 2 * K], f16)
                        nc.sync.dma_start(
                            gn2[:],
                            gn_d[p].rearrange("(ch n) kk -> n ch kk", n=NC2),
                        )
                        for ch in range(2):
                            aps = actps.tile([NC2, CD_W], f32)
                            nc.tensor.matmul(
                                aps[:],
                                gt2[:, NC2 * ch: NC2 * (ch + 1)],
                                cd[:],
                                start=True, stop=True,
                            )
                            act = act_tiles[(2 * p + ch) % 8]
                            nc.scalar.activation(
                                out=act.rearrange("p (c e) -> p c e", e=37)[:, :, :H],
                                in_=aps.rearrange("p (c e) -> p c e", e=H),
                                func=mybir.ActivationFunctionType.Sigmoid,
                            )
                            nc.tensor.matmul(
                                m_ps[64 * gam: 64 * (gam + 1),
                                     ACT_W * psi: ACT_W * (psi + 1)],
                                gn2[:, ch, :],
                                act[:],
                                start=(ch == 0), stop=(ch == 1),
                            )
                    prod = prodp.tile([128, 4 * ACT_W], f32)
                    nc.vector.tensor_mul(prod[:], m_ps[:], w2m_sb[:])
                    outt = outp.tile([128, 4], f32)
                    nc.vector.reduce_sum(
                        outt[:],
                        prod.rearrange("p (s e) -> p s e", e=ACT_W),
                        axis=mybir.AxisListType.X,
                    )
                    # out[4*ps+2*gm+cc, k] = outt[64*gm+32*cc+k, ps]; the DRAM
                    # offset is 128*ps + P so the DRAM AP collapses to [128, 4]
                    nc.sync.dma_start(
                        out_d[16 * mt: 16 * (mt + 1)].rearrange(
                            "(ps gm cc) k -> (gm cc k) ps", gm=2, cc=2
                        ),
                        outt[:],
                    )
    nc.compile()
    return nc


_NC_CACHE = None


def kernel(candidate_ad, goods, W1, b1, W2, b2):
    import concourse.bass_utils as bass_utils

    global _NC_CACHE
    gt, gn, candT1, wpack, w2m = _host_pack(
        candidate_ad.astype(np.float32), goods.astype(np.float32),
        W1.astype(np.float32), b1.astype(np.float32),
        W2.astype(np.float32), b2.astype(np.float32),
    )
    if _NC_CACHE is None:
        _NC_CACHE = _build_nc()
    nc = _NC_CACHE

    in_maps = []
    for i in range(NCORES):
        bs = slice(BL * i, BL * (i + 1))
        ps = slice(PAIRS * i, PAIRS * (i + 1))
        in_maps.append({
            "gt": gt[bs],
            "gn": gn[ps],
            "candT": np.ascontiguousarray(candT1[:, bs]),
            "wpack": wpack,
            "w2m": w2m,
        })
    res = bass_utils.run_bass_kernel_spmd(nc, in_maps, core_ids=list(range(NCORES)))
    out = np.concatenate([r["out"] for r in res.results], axis=0)
    return out.astype(np.float32)


if __name__ == "__main__":
    rng = np.random.default_rng(0)
    cand = rng.standard_normal((B, K)).astype(np.float32)
    goods = rng.standard_normal((B, N, K)).astype(np.float32)
    W1 = (rng.standard_normal((K * (K + 2), H)) / np.sqrt(K * (K + 2))).astype(np.float32)
    b1 = np.zeros(H, np.float32)
    W2 = (rng.standard_normal((H, 1)) / np.sqrt(H)).astype(np.float32)
    b2 = np.zeros(1, np.float32)
    out = kernel(candidate_ad=cand, goods=goods, W1=W1, b1=b1, W2=W2, b2=b2)
    print(out.shape, out.dtype)


# revision 7
# speedup vs baseline: 3.4324x; 3.4324x over previous
"""DIN attention-unit kernel for Trainium2 (8 NeuronCores, data-parallel over batch).

Math (per batch element b, N=200 items, K=32, HID=36):
    act   = sigmoid([k_b, q_{b,n}, k_b (x) q_{b,n}] @ W1 + b1)   # [N, 36]
    out_b = sum_n q_{b,n} * (act @ W2 + b2)[n]                   # [32]

Factorization: (k (x) q) . W1o = q . (k . W1o_r), so precompute per-b
CD_b = [[C_b],[d_b]] ([33, 36]) with C_b = W1q + k_b.W1o, d_b = k_b.W1k + b1;
act[n] = sigmoid([q_n, 1] @ CD_b); with M_b = [q|1]^T @ [act|1] over n,
out_b = M_b @ [W2; b2].

Device dataflow per core (128 b's = 64 pairs), all goods data fp16,
fp32 PSUM accumulation (end-to-end rel err ~3e-4):
  - Big slab DMAs only (DMA issue costs ~750ns fixed on the queueing engine):
    goods ships twice from host (transposed+ones [66, 64, 200] and natural
    pair-packed [100, 64, 256]), CD parks in DRAM once and reloads as a
    block-diagonal slab.
  - phase 1: one matmul candT_ones^T @ W_pack -> CD for all b.
  - per pair p, n-parity chunk ch: act-mm (stationary = goodsT+ones pair
    [66, 100], rhs = block-diag CD [66, 72]) -> natural pre-act [100, 72];
    sigmoid batched over 4 pairs; M-mm (stationary = natural goods
    [100, 64], rhs = [act|1] [100, 74]) -> [64, 74] PSUM blocks.
  - epilogue: batched DVE multiply by masked/replicated [W2; b2] + reduce.
"""

import sys

import numpy as np

if "/opt/trn_rl_repo" not in sys.path:
    sys.path.insert(0, "/opt/trn_rl_repo")

B, N, K, H = 1024, 200, 32, 36
NCORES = 8
BL = B // NCORES          # 128 batch elements per core
PAIRS = BL // 2           # 64
NC2 = N // 2              # 100, n-chunk size (n-parity chunking)
CD_W = 2 * H              # 72  (pair block-diag CD width)
ACT_W = 2 * 37            # 74  (act|ones pair width)
MT = 8                    # m-tiles per core (16 b's each)
CDW = (K + 1) * H         # 1188


def _host_pack(candidate_ad, goods, W1, b1, W2, b2):
    f16 = np.float16
    goods16 = goods.astype(f16)                        # [B, N, K]

    # transposed goods + ones row: gt[b, j, n]; j=32 row is ones
    gt = np.empty((B, K + 1, N), dtype=f16)
    gt[:, :K, :] = goods16.transpose(0, 2, 1)
    gt[:, K, :] = f16(1.0)

    # natural goods, pair-packed on the k axis: gn[p, n, 32c+k] = goods[2p+c, n, k]
    gn = np.empty((B // 2, N, 2 * K), dtype=f16)
    gn[:, :, :K] = goods16[0::2]
    gn[:, :, K:] = goods16[1::2]

    # candT with ones row: [33, B]
    candT1 = np.concatenate(
        [candidate_ad.T, np.ones((1, B), np.float32)], axis=0
    ).astype(np.float32)

    # W_pack [33, 1188]: row i (i<32): cols j*36+h = W1[2K + i*K + j, h]; cols 1152+h = W1[i, h]
    #                    row 32:      cols j*36+h = W1[K + j, h];        cols 1152+h = b1[h]
    wpack = np.empty((K + 1, CDW), np.float32)
    wpack[:K, : K * H] = W1[2 * K:].reshape(K, K * H)
    wpack[K, : K * H] = W1[K: 2 * K].reshape(K * H)
    wpack[:K, K * H:] = W1[:K]
    wpack[K, K * H:] = b1

    # masked replicated [W2; b2] for the DVE epilogue: [128, 296]
    w2b2 = np.concatenate([W2[:, 0], b2]).astype(np.float32)   # [37]
    base = np.zeros((64, ACT_W), np.float32)
    for c in range(2):
        base[32 * c: 32 * (c + 1), 37 * c: 37 * (c + 1)] = w2b2[None, :]
    w2m = np.tile(base, (2, 4))                        # [128, 296]
    return gt, gn, candT1, wpack, w2m


def _build_nc():
    import concourse.bacc as bacc
    import concourse.tile as tile
    from concourse import mybir

    f16 = mybir.dt.float16
    f32 = mybir.dt.float32

    nc = bacc.Bacc()
    gt_d = nc.dram_tensor("gt", [BL, K + 1, N], f16, kind="ExternalInput")
    gn_d = nc.dram_tensor("gn", [PAIRS, N, 2 * K], f16, kind="ExternalInput")
    candT_d = nc.dram_tensor("candT", [K + 1, BL], f32, kind="ExternalInput")
    wpack_d = nc.dram_tensor("wpack", [K + 1, CDW], f32, kind="ExternalInput")
    w2m_d = nc.dram_tensor("w2m", [128, 4 * ACT_W], f32, kind="ExternalInput")
    out_d = nc.dram_tensor("out", [BL, K], f32, kind="ExternalOutput")

    GSLAB = 4               # slab DMAs per goods tensor (pipelining granularity)
    PP = PAIRS // GSLAB     # 16 pairs per slab

    with tile.TileContext(nc) as tc:
        with (
            tc.tile_pool(name="const", bufs=1) as constp,
            tc.tile_pool(name="dram", bufs=1, space="DRAM") as dramp,
        ):
            wpack_sb = constp.tile([K + 1, CDW], f32)
            nc.sync.dma_start(wpack_sb[:], wpack_d[:])
            candT_sb = constp.tile([K + 1, BL], f32)
            nc.sync.dma_start(candT_sb[:], candT_d[:])
            w2m_sb = constp.tile([128, 4 * ACT_W], f32)
            nc.sync.dma_start(w2m_sb[:], w2m_d[:])

            # persistent goods slabs
            gt_all = constp.tile([2 * (K + 1), PAIRS, N], f16)      # [66, 64, 200]
            gn_all = constp.tile([NC2, PAIRS, 2, 2 * K], f16)       # [100, 64, 2, 64]
            for s in range(GSLAB):
                ps = slice(PP * s, PP * (s + 1))
                # gt[2p+c, j, n] -> gt_all[33c+j, p, n]
                nc.sync.dma_start(
                    gt_all[:, ps, :],
                    gt_d[2 * PP * s: 2 * PP * (s + 1)].rearrange(
                        "(p c) j n -> (c j) p n", c=2
                    ),
                )
                # gn[p, 2n'+ch, kk] -> gn_all[n', p, ch, kk]
                nc.scalar.dma_start(
                    gn_all[:, ps, :, :],
                    gn_d[ps].rearrange("p (n ch) kk -> n p ch kk", ch=2),
                )

            # ---- phase 1: CD for all 128 b's -> DRAM -> block-diag slab ----
            c_dram = dramp.tile([BL, CDW], f16)
            with tc.tile_pool(name="ph1psum", bufs=1, space="PSUM") as ph1p:
                c_ps = ph1p.tile([BL, CDW], f32)
                for lo, hi in ((0, 512), (512, 1024), (1024, CDW)):
                    nc.tensor.matmul(
                        c_ps[:, lo:hi], candT_sb[:], wpack_sb[:, lo:hi],
                        start=True, stop=True,
                    )
                c_sb = constp.tile([BL, CDW], f16)
                nc.vector.tensor_copy(c_sb[:], c_ps[:])
            nc.sync.dma_start(c_dram[:], c_sb[:])

            # cd_all[33c+j, p, 36c+h] = CD_{2p+c}[j, h], zeros off-diagonal
            cd_all = constp.tile([2 * (K + 1), PAIRS, CD_W], f16)   # [66, 64, 72]
            nc.gpsimd.memset(cd_all[:], 0.0)
            for c in range(2):
                nc.sync.dma_start(
                    cd_all[33 * c: 33 * (c + 1), :, 36 * c: 36 * (c + 1)],
                    c_dram[c::2].rearrange("p (j h) -> j p h", h=H),
                )

            with (
                tc.tile_pool(name="actp", bufs=1) as actp,
                tc.tile_pool(name="prodp", bufs=2) as prodp,
                tc.tile_pool(name="actps", bufs=3, space="PSUM") as actps,
                tc.tile_pool(name="mps", bufs=2, space="PSUM") as mps,
            ):
                out_sb = constp.tile([128, K], f32)
                # act tiles [100, 296] = 4 pairs x (36 act | 1) x 2 b's,
                # persistent with preset ones columns
                act_tiles = []
                for i in range(4):
                    t = actp.tile([NC2, 4 * ACT_W], f16, tag=f"act{i}")
                    tv = t.rearrange("p (g c e) -> p g c e", g=4, e=37)
                    nc.vector.memset(tv[:, :, :, 36:37], 1.0)
                    act_tiles.append(t)

                for mt in range(MT):
                    m_ps = mps.tile([128, 4 * ACT_W], f32)
                    for g4 in range(2):                 # 4-pair groups in this m-tile
                        p0 = 8 * mt + 4 * g4
                        acts = []
                        for ch in range(2):
                            apsb = actps.tile([NC2, 4 * CD_W], f32)
                            for pp in range(4):
                                p = p0 + pp
                                nc.tensor.matmul(
                                    apsb[:, CD_W * pp: CD_W * (pp + 1)],
                                    gt_all[:, p, ch::2],
                                    cd_all[:, p, :],
                                    start=True, stop=True,
                                )
                            act = act_tiles[(2 * (2 * mt + g4) + ch) % 4]
                            nc.scalar.activation(
                                out=act.rearrange(
                                    "p (g c e) -> p g c e", g=4, e=37
                                )[:, :, :, :H],
                                in_=apsb.rearrange("p (g e) -> p g e", e=H),
                                func=mybir.ActivationFunctionType.Sigmoid,
                            )
                            acts.append(act)
                        for pp in range(4):
                            p = p0 + pp
                            gam, psi = p % 2, (p // 2) % 4
                            for ch in range(2):
                                nc.tensor.matmul(
                                    m_ps[64 * gam: 64 * (gam + 1),
                                         ACT_W * psi: ACT_W * (psi + 1)],
                                    gn_all[:, p, ch, :],
                                    acts[ch][:, ACT_W * pp: ACT_W * (pp + 1)],
                                    start=(ch == 0), stop=(ch == 1),
                                )
                    prod = prodp.tile([128, 4 * ACT_W], f32)
                    nc.vector.tensor_mul(prod[:], m_ps[:], w2m_sb[:])
                    nc.vector.reduce_sum(
                        out_sb[:, 4 * mt: 4 * (mt + 1)],
                        prod.rearrange("p (s e) -> p s e", e=ACT_W),
                        axis=mybir.AxisListType.X,
                    )
                # out[16*mt + 4*ps + 2*gm + cc, k] = out_sb[64gm+32cc+k, 4mt+ps]
                # DRAM offset = 512*mt + 128*ps + P  -> AP [[1,128],[512,8],[128,4]]
                nc.sync.dma_start(
                    out_d.rearrange("(mt ps gm cc) k -> (gm cc k) mt ps", mt=MT, ps=4, gm=2),
                    out_sb.rearrange("p (mt ps) -> p mt ps", ps=4),
                )
    nc.compile()
    return nc


_NC_CACHE = None


def kernel(candidate_ad, goods, W1, b1, W2, b2):
    import concourse.bass_utils as bass_utils

    global _NC_CACHE
    gt, gn, candT1, wpack, w2m = _host_pack(
        candidate_ad.astype(np.float32), goods.astype(np.float32),
        W1.astype(np.float32), b1.astype(np.float32),
        W2.astype(np.float32), b2.astype(np.float32),
    )
    if _NC_CACHE is None:
        _NC_CACHE = _build_nc()
    nc = _NC_CACHE

    in_maps = []
    for i in range(NCORES):
        bs = slice(BL * i, BL * (i + 1))
        ps = slice(PAIRS * i, PAIRS * (i + 1))
        in_maps.append({
            "gt": gt[bs],
            "gn": gn[ps],
            "candT": np.ascontiguousarray(candT1[:, bs]),
            "wpack": wpack,
            "w2m": w2m,
        })
    res = bass_utils.run_bass_kernel_spmd(nc, in_maps, core_ids=list(range(NCORES)))
    out = np.concatenate([r["out"] for r in res.results], axis=0)
    return out.astype(np.float32)


if __name__ == "__main__":
    rng = np.random.default_rng(0)
    cand = rng.standard_normal((B, K)).astype(np.float32)
    goods = rng.standard_normal((B, N, K)).astype(np.float32)
    W1 = (rng.standard_normal((K * (K + 2), H)) / np.sqrt(K * (K + 2))).astype(np.float32)
    b1 = np.zeros(H, np.float32)
    W2 = (rng.standard_normal((H, 1)) / np.sqrt(H)).astype(np.float32)
    b2 = np.zeros(1, np.float32)
    out = kernel(candidate_ad=cand, goods=goods, W1=W1, b1=b1, W2=W2, b2=b2)
    print(out.shape, out.dtype)
